# revision 34
# baseline (speedup 1.0000x reference)
"""Trainium2 Bass kernel for nn_CompAttnSenseNet (self-contained).

Sharding: data-parallel over batch (mb=256 -> 32 examples on each of 8
NeuronCores); full 50k output projection per core (no collectives).

Per core (v2 design):
  - embedding table stored fp8e4 (x16 scale); token rows fetched with
    per-column indirect gathers (the only offset->output pairing that is
    well-defined on real HW) into E [pos, d] chunks, 4 examples per batch.
  - E chunks are PE-transposed (fp8 psum, element step 2 per the HW fp8
    transpose rule) into ET [d, pos]; psum drained by DVE/Act copies.
  - pos-contractions (gmean / context / hidden) run with E chunks as the
    *stationary* operand and tiny [128,1] moving vectors -> near-zero PE
    streaming cost; results land directly in [d, e] psum layout.
  - d-contractions (sense+q via an M=32 stationary window, sim likewise)
    stream ET once each; per-pass psum is consolidated through an SQ tile
    into dense [32 examples, pos] softmax layout with single-row DMAs.
  - softmax scale factors are fused into the Exp activations; elementwise
    stages are column-split across DVE and Pool.
  - output projection: W stored fp8 (x256) in a host-permuted padded tile
    layout, prefetched on SP during the frontend; logits land in
    [32j+e, 500] psum tiles, are scale-copied into one fin tile (bf16),
    Exp+accum produces per-row sumexp; log/lse-subtract happen on the
    host (f32), and the output leaves in ONE DMA (dst rows [3200, 512]).
PAD tokens need no masking: table[PAD] = 0 nullifies their contribution.
"""
import numpy as np

import concourse.bass as bass
import concourse.bacc as bacc
import concourse.mybir as mybir
import concourse.tile as tile
from concourse.bass_utils import run_bass_kernel_spmd

MB, L, S, D, V, O = 256, 200, 5, 128, 50000, 50000
NCORE = 8
BE = MB // NCORE          # 32 examples per core
LS = L * S                # 1000
LSP = 1024                # padded positions per example
NCH = LSP // 128          # 8 position chunks per example
NB = 8                    # gather batches / d-contract passes
NJ = 4                    # examples per pass (psum row-groups 32j)
TW = 500                  # logits tile width; vocab = 4 groups x 25 x 500
NT = 25
WCHUNK = 5                # logits tiles per W-load tensor

ESCALE = 16.0             # table fp8 scale
WQSCALE = 256.0           # w_attn fp8 scale
CSCALE = 16.0             # extra context fp8 scale
WSCALE = 256.0            # W_out fp8 scale
SENSE_SC = 1.0 / (ESCALE * ESCALE)        # sense_raw -> emb.gsum
Q_SC = 1.0 / (ESCALE * WQSCALE)           # q_raw -> q_true
SIM_SC = 1.0 / (ESCALE * ESCALE * CSCALE)  # sim_raw -> sim_true
HSCALE = 64.0             # hidden fp8 scale (for DoubleRow logits)
LOGIT_SC = 1.0 / (WSCALE * HSCALE)        # logits_raw -> logits_true

f32 = mybir.dt.float32
bf16 = mybir.dt.bfloat16
fp8 = mybir.dt.float8e4
i32 = mybir.dt.int32
np_bf16 = mybir.dt.np(bf16)
np_fp8 = mybir.dt.np(fp8)
FX = mybir.ActivationFunctionType
PM = mybir.MatmulPerfMode
ALU = mybir.AluOpType
AX = mybir.AxisListType

_cache = {}


def _bcast5(ap):
    """[P, L] AP -> [P, L, 5] with step-0 broadcast on the last dim."""
    return bass.AP(ap.tensor, ap.offset, list(ap.ap) + [[0, S]])


def _dk(ap):
    """moving AP -> [K, 2, N] double-k-tile view for DoubleRow. The second
    k-tile aliases the NEXT N columns (real stride; its data is multiplied
    by the zero half of the stationary, so any finite values work)."""
    n = ap.ap[1][1]
    return bass.AP(ap.tensor, ap.offset,
                   [list(ap.ap[0]), [n, 2], [1, n]])


def _tall(ap, inner):
    """Full-tile flat view as [N/inner, inner]: big first dim (cheap in the
    DMA cost model), contiguous last dim. Only valid for whole dense tiles."""
    n = ap.size()
    assert ap.offset == 0 and n % inner == 0
    return bass.AP(ap.tensor, 0, [[inner, n // inner], [1, inner]])


def build(b_attn: float, use_mask: bool, use_bout: bool):
    nc = bacc.Bacc(None, target_bir_lowering=False, debug=False)
    table = nc.dram_tensor("table", [V, D], fp8, kind="ExternalInput")
    idxT_d = nc.dram_tensor("idxT", [128, NB * 32], i32, kind="ExternalInput")
    wout_d = [
        nc.dram_tensor(f"wout{i}", [128, WCHUNK * 4 * TW + 250], fp8,
                       kind="ExternalInput")
        for i in range(NT // WCHUNK)
    ]
    id16_d = nc.dram_tensor("id16", [128, 128], bf16, kind="ExternalInput")
    id8_d = nc.dram_tensor("id8", [128, 128], fp8, kind="ExternalInput")
    ones_d = nc.dram_tensor("ones16", [128, 1], bf16, kind="ExternalInput")
    wq_d = nc.dram_tensor("wq8", [128, 1], fp8, kind="ExternalInput")
    lws_d = nc.dram_tensor("lws", [BE, 1], f32, kind="ExternalInput")
    lwsaw_d = nc.dram_tensor("lwsaw", [BE, 1], f32, kind="ExternalInput")
    mask_d = nc.dram_tensor("maskneg", [BE, L], f32, kind="ExternalInput")
    bout_d = nc.dram_tensor("bout", [128, NT * TW], f32, kind="ExternalInput")
    out_d = nc.dram_tensor("out", [BE * 100, 512], bf16, kind="ExternalOutput")
    sume_d = nc.dram_tensor("sume", [128, 1], f32, kind="ExternalOutput")

    alt = [0]

    def copy2(out_ap, in_ap):
        """rotate psum-drain copies over DVE / Act (Pool is gather-bound)"""
        eng = (nc.vector.tensor_copy, nc.scalar.copy)[alt[0] % 2]
        alt[0] += 1
        eng(out=out_ap, in_=in_ap)

    dmaq = [0]

    def dmae():
        e = (nc.sync, nc.scalar, nc.gpsimd)[dmaq[0] % 3]
        dmaq[0] += 1
        return e

    with tile.TileContext(nc) as tc:
        with (
            tc.tile_pool(name="const", bufs=1) as cp,
            tc.tile_pool(name="emb", bufs=1) as ep,
            tc.tile_pool(name="wpool", bufs=1) as wpl,
            tc.tile_pool(name="work", bufs=1) as wk,
            tc.tile_pool(name="ptp", bufs=2, space="PSUM") as ptp,
            tc.tile_pool(name="pqp", bufs=2, space="PSUM") as pqp,
            tc.tile_pool(name="smp", bufs=1, space="PSUM") as smp,
        ):
            # ---- constants (idxT first: gather depends on it)
            idx_t = cp.tile([128, NB * 32], i32, name="c_idx")
            nc.sync.dma_start(out=idx_t[:], in_=idxT_d[:])
            id16 = cp.tile([128, 128], bf16, name="c_id16")
            nc.sync.dma_start(out=id16[:], in_=id16_d[:])
            id8 = cp.tile([128, 128], fp8, name="c_id8")
            nc.sync.dma_start(out=id8[:], in_=id8_d[:])
            ones16 = cp.tile([128, 1], bf16, name="c_ones")
            nc.sync.dma_start(out=ones16[:], in_=ones_d[:])
            wq8 = cp.tile([128, 1], fp8, name="c_wq")
            nc.sync.dma_start(out=wq8[:], in_=wq_d[:])
            lws = cp.tile([BE, 1], f32, name="c_lws")
            nc.sync.dma_start(out=lws[:], in_=lws_d[:])
            lwsaw = cp.tile([BE, 1], f32, name="c_lwsaw")
            nc.sync.dma_start(out=lwsaw[:], in_=lwsaw_d[:])
            maskneg = None
            if use_mask:
                maskneg = cp.tile([BE, L], f32, name="c_mask")
                nc.sync.dma_start(out=maskneg[:], in_=mask_d[:])
            bout_t = None
            if use_bout:
                bout_t = cp.tile([128, NT * TW], f32, name="c_bout")
                nc.sync.dma_start(out=bout_t[:], in_=bout_d[:])

            # ---- W prefetch (5 flat loads; consumed in logits phase)
            wts = []
            wengs = [nc.sync, nc.sync, nc.sync, nc.sync, nc.sync]
            for i in range(NT // WCHUNK):
                wt = wpl.tile([128, WCHUNK * 4 * TW + 250], fp8,
                              name=f"wt{i}")
                wengs[i].dma_start(out=wt[:], in_=wout_d[i][:])
                wts.append(wt)

            # ---- gather: batch b holds examples {NB*j + b}, k = b*32+j*8+c
            Etiles = []
            for b in range(NB):
                Eb = ep.tile([128, 32 * 128], fp8, name=f"E{b}")
                # per-column gathers: the only indirect-DMA shape whose
                # offset->output pairing is well-defined on real hardware
                for kk in range(32):
                    k = b * 32 + kk
                    nc.gpsimd.indirect_dma_start(
                        out=Eb[:, kk * 128:(kk + 1) * 128],
                        out_offset=None,
                        in_=table[:],
                        in_offset=bass.IndirectOffsetOnAxis(
                            ap=idx_t[:, k:k + 1], axis=0
                        ),
                    )
                Etiles.append(Eb)

            def Ech(e, c):
                b, j = e % NB, e // NB
                k = j * NCH + c
                return Etiles[b][:, k * 128:(k + 1) * 128]

            # ---- per-batch frontend: transposes -> ET; gmean; gw fill;
            #      sense+q d-contract pass
            ET = ep.tile([128, BE * LSP + 256], fp8, name="ET")
            nc.vector.memset(ET[:, BE * LSP:], 0.0)
            # one psum bank holds all small accumulators:
            # cols [0,32) gmean | [32,64) context | [64,96) hidden | 96 pr
            acc = smp.tile([128, 128], f32, tag="acc", bufs=1)
            GM = acc
            # stationary blocks of 64: [gmean_e, wq, 30 junk | 32 zeros]
            # (second half = zero k-tile for DoubleRow)
            gw = wk.tile([128, 64 * BE], fp8, tag="gw")
            nc.vector.memset(gw[:], 0.0)
            gwq = gw[:].rearrange("p (e z) -> p e z", z=64)[:, :, 1]
            nc.vector.tensor_copy(out=gwq, in_=wq8[:].to_broadcast([128, BE]))

            SQ = wk.tile([128, NB * LSP], bf16, tag="sq")
            sense = wk.tile([BE, LSP], bf16, tag="sense")
            qall = wk.tile([BE, LSP], bf16, tag="qall")

            def emit_dpass(b):
                # sense+q pass b: e = j*NB + b -> psum rows (32j, 32j+1)
                pq = pqp.tile([128, LSP], f32, tag="pq")
                for j in range(NJ):
                    e = j * NB + b
                    for h in range(2):
                        nc.tensor.matmul(
                            out=pq[32 * j:32 * (j + 1), h * 512:(h + 1) * 512],
                            lhsT=gw[:, 64 * e:64 * e + 32],
                            rhs=ET[:, e * LSP + h * 512:
                                   e * LSP + (h + 1) * 512],
                            start=True, stop=True, tile_position=(0, 32 * j),
                        )
                copy2(SQ[:, b * LSP:(b + 1) * LSP], pq[:])

            for b in range(NB):
                for j in range(NJ):
                    e = j * NB + b
                    # transpose chunks of example e into one psum tile.
                    # HW fp8 transpose writes psum with element step 2, so
                    # the tile holds 2048 slots and every other byte is used.
                    pt = ptp.tile([128, 2 * LSP], fp8, tag="pt")
                    pta = pt[:]
                    for c in range(NCH):
                        nc.tensor.transpose(
                            out=bass.AP(pta.tensor, pta.offset + 256 * c,
                                        [list(pta.ap[0]), [2, 128]]),
                            in_=Ech(e, c), identity=id8[:],
                        )
                    if b == NB - 1:
                        ceng = (nc.vector.tensor_copy, nc.scalar.copy,
                                nc.vector.tensor_copy, nc.scalar.copy)[j]
                    else:
                        ceng = (nc.vector.tensor_copy,
                                nc.scalar.copy)[alt[0] % 2]
                        alt[0] += 1
                    ceng(out=ET[:, e * LSP:(e + 1) * LSP],
                         in_=bass.AP(pta.tensor, pta.offset,
                                     [list(pta.ap[0]), [2, LSP]]))
                    # gmean pos-contract (stationary = Ech)
                    for c in range(NCH):
                        nc.tensor.matmul(
                            out=GM[:, e:e + 1], lhsT=Ech(e, c),
                            rhs=ones16[:], start=(c == 0),
                            stop=(c == NCH - 1),
                        )
                # gw even cols for this batch's examples ({j*NB+b})
                src = GM[:, :BE].rearrange("p (j r) -> p j r", r=NB)[:, :, b]
                dst = gw[:].rearrange("p (j r z) -> p j r z",
                                      r=NB, z=64)[:, :, b, 0]
                nc.vector.tensor_copy(out=dst, in_=src)
                # d-pass of the PREVIOUS batch: keeps the in-order PE queue
                # from stalling on this batch's ET copy / gw fill
                if b >= 1:
                    emit_dpass(b - 1)
            emit_dpass(NB - 1)

            # ---- consolidation: SQ row 32j+m -> dense rows [NB*j, NB*(j+1))
            for j in range(NJ):
                src_s = SQ[32 * j:32 * j + 1, :].rearrange(
                    "p (g x) -> p g x", x=LSP)
                src_q = SQ[32 * j + 1:32 * j + 2, :].rearrange(
                    "p (g x) -> p g x", x=LSP)
                dmae().dma_start(out=sense[NB * j:NB * (j + 1), :], in_=src_s)
                dmae().dma_start(out=qall[NB * j:NB * (j + 1), :], in_=src_q)

            # ---- softmax helpers ------------------------------------------
            HL, HC = L // 2, LS // 2   # column halves (word-aligned)

            def grouped_softmax(src, dst, scale, post=None):
                """dst = softmax over S within words of src[:, :LS].
                Elementwise stages column-split over DVE (lo) / Pool (hi);
                the softmax scale is fused into the Exp activation."""
                ex = wk.tile([BE, LSP], bf16, tag="ex_sm")
                nc.scalar.activation(out=ex[:, :LS], in_=src[:, :LS],
                                     func=FX.Exp, scale=scale)
                sm = wk.tile([BE, 256], bf16, tag="sum_sm")
                with nc.allow_low_precision(reason="sum of 5 exps; 0.4% rel "
                                            "noise on softmax denominators"):
                    nc.vector.tensor_reduce(
                        out=sm[:, :L],
                        in_=ex[:, :LS].rearrange("p (l s) -> p l s", s=S),
                        axis=AX.X, op=ALU.add,
                    )
                with nc.allow_low_precision(reason="bf16 softmax denom"):
                    nc.vector.reciprocal(out=sm[:, :L], in_=sm[:, :L])
                if post is not None:
                    nc.vector.tensor_scalar_mul(
                        out=sm[:, :L], in0=sm[:, :L], scalar1=post)
                nc.gpsimd.memset(dst[:, LS:], 0.0)
                nc.vector.tensor_tensor(
                    out=dst[:, :HC].rearrange("p (l s) -> p l s", s=S),
                    in0=ex[:, :HC].rearrange("p (l s) -> p l s", s=S),
                    in1=_bcast5(sm[:, :HL]), op=ALU.mult,
                )
                nc.gpsimd.tensor_tensor(
                    out=dst[:, HC:LS].rearrange("p (l s) -> p l s", s=S),
                    in0=ex[:, HC:LS].rearrange("p (l s) -> p l s", s=S),
                    in1=_bcast5(sm[:, HL:L]), op=ALU.mult,
                )

            def vec_transpose(src, dst):
                """[BE, LSP] -> [128, (c, e)]: dst col c*BE+e."""
                pv = smp.tile([128, NCH * BE], bf16, tag="vt", bufs=1)
                for c in range(NCH):
                    nc.tensor.transpose(
                        out=pv[:, c * BE:(c + 1) * BE],
                        in_=src[:, c * 128:(c + 1) * 128],
                        identity=id16[:BE, :BE],
                    )
                nc.vector.tensor_copy(out=dst[:], in_=pv[:])

            # ---- sense softmax (scale folds lw/S and fp8 scales)
            sw = wk.tile([BE, LSP], bf16, tag="sw")
            grouped_softmax(sense, sw, lws[:])

            # ---- word attention (column-split DVE / Pool)
            wprod = wk.tile([BE, LSP], bf16, tag="wprod")
            nc.vector.tensor_tensor(
                out=wprod[:, :HC], in0=sw[:, :HC], in1=qall[:, :HC],
                op=ALU.mult)
            nc.gpsimd.tensor_tensor(
                out=wprod[:, HC:LS], in0=sw[:, HC:LS], in1=qall[:, HC:LS],
                op=ALU.mult)
            wimp = wk.tile([BE, 256], bf16, tag="wimp")
            with nc.allow_low_precision(reason="sum of 5 bf16 terms feeding "
                                        "the word softmax; noise ~0.4%"):
                nc.vector.tensor_reduce(
                    out=wimp[:, :L],
                    in_=wprod[:, :LS].rearrange("p (l s) -> p l s", s=S),
                    axis=AX.X, op=ALU.add,
                )
            if use_mask:
                nc.vector.tensor_tensor(
                    out=wimp[:, :L], in0=wimp[:, :L], in1=maskneg[:],
                    op=ALU.add)
            ew = wk.tile([BE, 256], f32, tag="ew")
            nc.scalar.activation(out=ew[:, :L], in_=wimp[:, :L], func=FX.Exp,
                                 bias=float(b_attn), scale=Q_SC)
            wsum = wk.tile([BE, 1], f32, tag="wsum")
            nc.vector.tensor_reduce(out=wsum[:], in_=ew[:, :L], axis=AX.X,
                                    op=ALU.add)
            nc.vector.reciprocal(out=wsum[:], in_=wsum[:])
            ww = wk.tile([BE, 256], f32, tag="ww")
            nc.vector.tensor_scalar_mul(out=ww[:, :L], in0=ew[:, :L],
                                        scalar1=wsum[:])

            # ---- u = word_w (x) sense_w; transpose to [128, (c, e)]
            u = wk.tile([BE, LSP], bf16, tag="u")
            nc.gpsimd.memset(u[:, LS:], 0.0)
            nc.vector.tensor_tensor(
                out=u[:, :HC].rearrange("p (l s) -> p l s", s=S),
                in0=sw[:, :HC].rearrange("p (l s) -> p l s", s=S),
                in1=_bcast5(ww[:, :HL]), op=ALU.mult,
            )
            nc.gpsimd.tensor_tensor(
                out=u[:, HC:LS].rearrange("p (l s) -> p l s", s=S),
                in0=sw[:, HC:LS].rearrange("p (l s) -> p l s", s=S),
                in1=_bcast5(ww[:, HL:L]), op=ALU.mult,
            )
            uT = wk.tile([128, NCH * BE], bf16, tag="uT")
            vec_transpose(u, uT)

            # ---- context pos-contract (stationary = Ech)
            for e in range(BE):
                for c in range(NCH):
                    nc.tensor.matmul(
                        out=acc[:, BE + e:BE + e + 1], lhsT=Ech(e, c),
                        rhs=uT[:, c * BE + e:c * BE + e + 1],
                        start=(c == 0), stop=(c == NCH - 1),
                    )
            ctx8 = wk.tile([128, 64 * BE], fp8, tag="ctx8")
            nc.gpsimd.memset(ctx8[:], 0.0)
            nc.vector.tensor_scalar_mul(
                out=ctx8[:].rearrange("p (e z) -> p e z", z=64)[:, :, 0],
                in0=acc[:, BE:2 * BE], scalar1=CSCALE)

            # ---- sim d-contract (M=1) + consolidation
            SQ2 = wk.tile([128, NB * LSP], bf16, tag="sq2")
            sim = wk.tile([BE, LSP], bf16, tag="sim")
            for b in range(NB):
                pq = pqp.tile([128, LSP], f32, tag="pq")
                for j in range(NJ):
                    e = j * NB + b
                    for h in range(2):
                        nc.tensor.matmul(
                            out=pq[32 * j:32 * (j + 1), h * 512:(h + 1) * 512],
                            lhsT=ctx8[:, 64 * e:64 * e + 32],
                            rhs=ET[:, e * LSP + h * 512:
                                   e * LSP + (h + 1) * 512],
                            start=True, stop=True, tile_position=(0, 32 * j),
                        )
                copy2(SQ2[:, b * LSP:(b + 1) * LSP], pq[:])
            for j in range(NJ):
                src_s = SQ2[32 * j:32 * j + 1, :].rearrange(
                    "p (g x) -> p g x", x=LSP)
                dmae().dma_start(out=sim[NB * j:NB * (j + 1), :], in_=src_s)

            # ---- attention softmax; hidden pos-contract
            aw = wk.tile([BE, LSP], bf16, tag="aw")
            grouped_softmax(sim, aw, SIM_SC, post=lwsaw[:])
            awT = wk.tile([128, NCH * BE], bf16, tag="awT")
            vec_transpose(aw, awT)

            for e in range(BE):
                for c in range(NCH):
                    nc.tensor.matmul(
                        out=acc[:, 2 * BE + e:2 * BE + e + 1], lhsT=Ech(e, c),
                        rhs=awT[:, c * BE + e:c * BE + e + 1],
                        start=(c == 0), stop=(c == NCH - 1),
                    )
            hid2 = wk.tile([128, 2 * BE], fp8, tag="hid2")
            nc.vector.memset(hid2[:, BE:], 0.0)
            nc.vector.tensor_scalar_mul(out=hid2[:, :BE],
                                        in0=acc[:, 2 * BE:3 * BE],
                                        scalar1=HSCALE)

            # ---- logits: psum row 32j+e , vocab col j*OG + t*TW + x
            fin = wk.tile([128, NT * TW], bf16, tag="fin")
            sacc = wk.tile([128, 32], f32, tag="sacc")
            etile = wk.tile([128, 2 * TW], bf16, tag="etile")
            for t in range(NT):
                wt = wts[t // WCHUNK]
                base = (t % WCHUNK) * 4 * TW
                pl = pqp.tile([128, LSP], f32, tag="pq")
                for j in range(4):
                    nc.tensor.matmul(
                        out=pl[32 * j:32 * (j + 1), :TW],
                        lhsT=hid2[:, :BE],
                        rhs=wt[:, base + j * TW:base + (j + 1) * TW],
                        start=True, stop=True, tile_position=(0, 32 * j),
                    )
                if use_bout:
                    nc.vector.tensor_tensor(
                        out=fin[:, t * TW:(t + 1) * TW],
                        in0=pl[:, :TW],
                        in1=bout_t[:, t * TW:(t + 1) * TW],
                        op=ALU.add,
                    )  # bout pre-scaled by WSCALE on host
                    nc.vector.tensor_scalar_mul(
                        out=fin[:, t * TW:(t + 1) * TW],
                        in0=fin[:, t * TW:(t + 1) * TW], scalar1=LOGIT_SC)
                else:
                    nc.vector.tensor_scalar_mul(
                        out=fin[:, t * TW:(t + 1) * TW], in0=pl[:, :TW],
                        scalar1=LOGIT_SC)
                # exp+accumulate over PAIRS of tiles (halves the per-op
                # activation-engine fixed costs); last (odd) tile alone
                if t % 2 == 1:
                    nc.scalar.activation(
                        out=etile[:], in_=fin[:, (t - 1) * TW:(t + 1) * TW],
                        func=FX.Exp, accum_out=sacc[:, t // 2:t // 2 + 1])
                elif t == NT - 1:
                    nc.scalar.activation(
                        out=etile[:, :TW], in_=fin[:, t * TW:(t + 1) * TW],
                        func=FX.Exp, accum_out=sacc[:, NT // 2:NT // 2 + 1])

            # ---- lse -> nls ; final add ; single output DMA
            s4 = wk.tile([128, 1], f32, tag="s4")
            nc.vector.tensor_reduce(out=s4[:], in_=sacc[:, :NT // 2 + 1],
                                    axis=AX.X, op=ALU.add)
            nc.scalar.dma_start(out=sume_d[:], in_=s4[:])

            # out row p*NT + t (p = psum partition 32j+e); host decodes.
            # dst rows padded to 512 so dims can't be merged into one flat
            # run (which would defeat the [3200, 500] descriptor shape).
            nc.sync.dma_start(
                out=out_d[:, :TW],
                in_=fin[:].rearrange("p (t x) -> p t x", x=TW))
    nc.compile()
    return nc


def host_inputs(inputs, length_weights, word_attn_mask, embedding, W_out,
                b_out, w_attn):
    table = (np.asarray(embedding, np.float32) * ESCALE).astype(np_fp8)
    # W permuted: wout[i][d, s*2000 + j*500 + x] = W[d, j*12500 + (5i+s)*500+x]
    W = np.asarray(W_out, np.float32).reshape(D, 4, NT, TW)  # [d, j, t, x]
    Wp = (W.transpose(0, 2, 1, 3) * WSCALE).astype(np_fp8)   # [d, t, j, x]
    id16 = np.eye(128, dtype=np.float32).astype(np_bf16)
    id8 = np.eye(128, dtype=np.float32).astype(np_fp8)
    ones16 = np.ones((128, 1), np.float32).astype(np_bf16)
    wq8 = (np.asarray(w_attn, np.float32).reshape(D, 1) * WQSCALE).astype(
        np_fp8)
    lw = np.asarray(length_weights, np.float32)[:, 0, 0]
    idx = np.asarray(inputs).astype(np.int64)
    mask = np.asarray(word_attn_mask)
    bout = np.asarray(b_out, np.float32)

    in_maps = []
    for kcore in range(NCORE):
        sl = slice(kcore * BE, (kcore + 1) * BE)
        idx_pad = np.zeros((BE, LSP), np.int32)
        idx_pad[:, :LS] = idx[sl]
        arr = idx_pad.reshape(BE, NCH, 128)          # [e, c, p]
        # k = b*32 + j*8 + c  with  e = j*NB + b
        kcols = np.stack([arr[j * NB + b, c]
                          for b in range(NB)
                          for j in range(NJ)
                          for c in range(NCH)])       # [256, 128]
        idxT = np.ascontiguousarray(kcols.T, np.int32)  # [128, 256]
        lw_k = lw[sl]
        m = {
            "table": table,
            "idxT": idxT,
            "id16": id16,
            "id8": id8,
            "ones16": ones16,
            "wq8": wq8,
            "lws": (lw_k / S * SENSE_SC).reshape(BE, 1).astype(np.float32),
            "lwsaw": (lw_k / ESCALE).reshape(BE, 1).astype(np.float32),
            "maskneg": np.where(mask[sl], -1e30, 0.0).astype(np.float32),
            "bout": _bout_tile(bout),
        }
        for i in range(NT // WCHUNK):
            wchunk = Wp[:, i * WCHUNK:(i + 1) * WCHUNK].reshape(
                D, WCHUNK * 4 * TW)
            wpad = np.zeros((D, WCHUNK * 4 * TW + 250), np_fp8)
            wpad[:, :WCHUNK * 4 * TW] = wchunk
            m[f"wout{i}"] = wpad
        in_maps.append(m)
    return in_maps


def _bout_tile(bout):
    """[128, NT*TW] f32: row 32j+e, col t*TW+x = b_out[j*OG + t*TW + x]*WSC."""
    b = np.asarray(bout, np.float32).reshape(4, NT, TW)     # [j, t, x]
    t = np.zeros((128, NT * TW), np.float32)
    for j in range(4):
        t[32 * j:32 * (j + 1), :] = (b[j] * WSCALE).reshape(1, NT * TW)
    return t


def kernel(**inputs):
    b_attn = float(np.asarray(inputs["b_attn"], np.float32))
    use_mask = bool(np.asarray(inputs["word_attn_mask"]).any())
    use_bout = bool(np.any(np.asarray(inputs["b_out"]) != 0))
    key = (use_mask, use_bout, round(b_attn, 9))
    if key not in _cache:
        _cache[key] = build(b_attn, use_mask, use_bout)
    nc = _cache[key]
    in_maps = host_inputs(
        inputs["inputs"], inputs["length_weights"], inputs["word_attn_mask"],
        inputs["embedding"], inputs["W_out"], inputs["b_out"],
        inputs["w_attn"],
    )
    res = run_bass_kernel_spmd(nc, in_maps, list(range(NCORE)))
    outs = []
    for k in range(NCORE):
        r = res.results[k]["out"].astype(np.float32)[:, :TW]  # [3200, 500]
        sume = res.results[k]["sume"].astype(np.float32).reshape(4, BE)
        lse = np.log(sume.sum(axis=0))                         # [BE]
        # row (32j+e)*NT + t, col x  ->  out[e, j*OG + t*TW + x]
        r = r.reshape(4, BE, NT, TW) - lse[None, :, None, None]
        outs.append(r.transpose(1, 0, 2, 3).reshape(BE, O))
    return np.concatenate(outs, axis=0)
